# revision 3
# baseline (speedup 1.0000x reference)
"""EdgeBlock GNN kernel v2 for 8 Trainium2 NeuronCores.

Sharding: edges split into 8 shards of 50000; triplets assigned to the core
owning index_ji; inside a core, 3 thirds so dma_gather indices fit int16.

v2 changes vs baseline:
- fp16 single-precision data path (rel-err budget 2e-2 >> fp16's ~1e-3):
  one matmul per input slice instead of hi/lo triple, 256B gather rows.
- kj pre-phase (bucket gather + scatter reorder, ~1ms serial) replaced by
  per-chunk indirect_dma_start with int32 indices (128 rows/op) + PE
  transpose.
- Scalar engine uses only the sigmoid_and_others act set (Sigmoid/Tanh/
  Copy); LayerNorm sqrt is batched per block ([P, 21] strided) so act-table
  reloads drop from ~1850 to ~100.
- msg_s stored fp16 (halves phase-B gather bytes).

Per core, per third q:
  phase A: per block (21 chunks): transposed fp16 dma_gathers of
    node_i/j/k and e_ji; per chunk indirect e_kj gather + PE transpose;
    z = 5 fp16 matmuls -> [128,256] f32 PSUM; bn_stats -> batched
    sqrt/recip; sigmoid/tanh with fused (x-m)*rs; msg = sg*th fp16;
    selection-matrix matmul combines equal-ji runs; msum -> msg_s fp16.
  phase B: per block: msgt gather (first-of-run rows = segment sums),
    node_i/j transposed gathers; prodT = ni*nj feature-major; z2 matmul;
    batched LN; sigmoid/tanh; c2p LN + c3 LN (batched sqrt); out =
    tanh(edge + c2emb + c3emb) f32.
"""
import sys
for _p in ("/opt/trn_rl_repo", "/root/.axon_site/_ro/trn_rl_repo"):
    if _p not in sys.path:
        sys.path.insert(0, _p)

import numpy as np

P = 128
N_NODES, N_EDGES, N_TRIP = 20000, 400000, 500000
DN = DE = 128
N_CORES = 8
E_SH = N_EDGES // N_CORES            # 50000
Q = 3
E_THIRD = [16667, 16667, 16666]
E_OFF = [0, 16667, 33334]
ECH = 131                            # edge chunks per third
E_CAP = ECH * P                      # 16768
T_CH = 168                           # triplet chunks per third
T_CAP = T_CH * P                     # 21504
B_CH = 21                            # chunks per phase-A block
B_T = B_CH * P                       # 2688
NBLK = T_CH // B_CH                  # 8
MSG_ROWS = T_CAP + P                 # 21632
ZERO_ROW = T_CAP
EPS = 1e-5
EB_CH = [21] * 6 + [5]               # phase-B block chunk counts (=131)
E_SH_PAD = 50176                     # padded shard rows (covers E_OFF[2]+E_CAP)


def _wrap16(vals, cap):
    """[cap] int array -> [128, cap/16] wrapped int16 (replicated 8x)."""
    assert cap % 16 == 0 and vals.shape[0] == cap
    assert vals.min() >= 0 and vals.max() <= 32767, (vals.min(), vals.max())
    w = np.zeros((16, cap // 16), np.int16)
    w[np.arange(cap) % 16, np.arange(cap) // 16] = vals.astype(np.int16)
    return np.tile(w, (8, 1))


def _pack_runs(ji_loc):
    """Positions for sorted ji_loc so no equal-value run crosses a 128
    boundary. Returns (pos array, total_padded_len)."""
    n = ji_loc.shape[0]
    starts = np.flatnonzero(np.r_[True, ji_loc[1:] != ji_loc[:-1]])
    lens = np.diff(np.r_[starts, n])
    assert lens.max() <= P, f"run length {lens.max()} > 128"
    pos = np.empty(n, np.int64)
    cur = 0
    for s, l in zip(starts, lens):
        if (cur % P) + l > P:
            cur = (cur // P + 1) * P
        pos[s:s + l] = cur + np.arange(l)
        cur += l
    return pos, ((cur + P - 1) // P) * P


def _prep_core(m, i, j, idx_i, idx_j, idx_k, ji, kj, trips_sorted, ji_sorted):
    lo = np.searchsorted(ji_sorted, m * E_SH, "left")
    hi = np.searchsorted(ji_sorted, (m + 1) * E_SH, "left")
    trips_m = trips_sorted[lo:hi]
    ji_m = ji_sorted[lo:hi]

    gi = np.zeros((Q, P, T_CAP // 16), np.int16)
    gj = np.zeros((Q, P, T_CAP // 16), np.int16)
    gk = np.zeros((Q, P, T_CAP // 16), np.int16)
    gkj32 = np.zeros((Q, P, T_CH), np.int32)
    jiv = np.zeros((Q, T_CH, P), np.float16)
    ci = np.zeros((Q, P, T_CAP // 16), np.int16)
    cj = np.zeros((Q, P, T_CAP // 16), np.int16)
    emap = np.full((Q, T_CAP), -1, np.int64)

    for q in range(Q):
        base = m * E_SH + E_OFF[q]
        qlo = np.searchsorted(ji_m, base, "left")
        qhi = np.searchsorted(ji_m, base + E_THIRD[q], "left")
        t = trips_m[qlo:qhi]                     # triplet ids, ji ascending
        jil = ji_m[qlo:qhi] - base               # [0, E_THIRD[q])
        pos, used = _pack_runs(jil)
        assert used <= T_CAP, f"third overflow {used} > {T_CAP}"

        ai = np.zeros(T_CAP, np.int64)
        aj = np.zeros(T_CAP, np.int64)
        ak = np.zeros(T_CAP, np.int64)
        akj = np.zeros(T_CAP, np.int64)
        ai[pos] = idx_i[t]
        aj[pos] = idx_j[t]
        ak[pos] = idx_k[t]
        akj[pos] = kj[t]
        gi[q] = _wrap16(ai, T_CAP)
        gj[q] = _wrap16(aj, T_CAP)
        gk[q] = _wrap16(ak, T_CAP)
        # slot s = c*128 + p  ->  gkj32[p, c]
        gkj32[q] = akj.reshape(T_CH, P).T.astype(np.int32)

        # E' renumbering: each run's edge gets the next edge slot of its
        # chunk; zero-triplet edges fill spare slots afterwards.
        rstart = np.flatnonzero(np.r_[True, jil[1:] != jil[:-1]])
        rpos = pos[rstart]                     # first-of-run slot
        rchunk = rpos // P                     # chunk of each run
        redge = jil[rstart]                    # third-local edge id
        eslot = np.full(T_CAP, -1, np.int64)   # E' slot -> edge id
        nrun = np.zeros(T_CH, np.int64)
        rslot = np.empty(rstart.shape[0], np.int64)
        for rr in range(rstart.shape[0]):
            c = rchunk[rr]
            rslot[rr] = c * P + nrun[c]
            nrun[c] += 1
        eslot[rslot] = redge
        # jiw per triplet slot: edge-slot index of its run within chunk
        jiw = np.full(T_CAP, -1.0, np.float16)
        runid = np.cumsum(np.r_[0, (jil[1:] != jil[:-1]).astype(np.int64)])
        jiw[pos] = (rslot[runid] % P).astype(np.float16)
        jiv[q] = jiw.reshape(T_CH, P)

        has_t = np.zeros(E_THIRD[q], bool)
        has_t[redge] = True
        zfree = np.flatnonzero(~has_t)         # zero-triplet edges
        spare = np.flatnonzero(eslot < 0)
        spare = spare[np.argsort(nrun[spare // P] * 0 + spare)]  # ascending
        eslot[spare[:zfree.shape[0]]] = zfree
        emap[q] = eslot

        bi = np.zeros(T_CAP, np.int64)
        bj = np.zeros(T_CAP, np.int64)
        val = eslot >= 0
        bi[val] = i[base + eslot[val]]
        bj[val] = j[base + eslot[val]]
        ci[q] = _wrap16(bi, T_CAP)
        cj[q] = _wrap16(bj, T_CAP)

    return dict(gidx_i=gi, gidx_j=gj, gidx_k=gk,
                gkj32=gkj32, jiv=jiv,
                cidx_i=ci, cidx_j=cj), emap


_CACHE = {}


def _build_kernel(fast):
    import concourse.bass as bass
    import concourse.bacc as bacc
    import concourse.tile as tile
    from concourse import mybir
    from concourse.masks import make_identity

    f32 = mybir.dt.float32
    fp16 = mybir.dt.float16
    i16 = mybir.dt.int16
    i32 = mybir.dt.int32
    AF = mybir.ActivationFunctionType
    OP = mybir.AluOpType

    nc = bacc.Bacc("TRN2")
    node_h = nc.dram_tensor("node_h", [N_NODES, DN], fp16, kind="ExternalInput")
    edge_h = nc.dram_tensor("edge_h", [N_EDGES, DE], fp16, kind="ExternalInput")
    eprime = nc.dram_tensor("eprime", [Q, T_CAP, DE], fp16, kind="ExternalInput")
    eprimeT = nc.dram_tensor("eprimeT", [Q, DN, T_CAP], fp16, kind="ExternalInput")
    iotaf = nc.dram_tensor("iotaf", [P, P], fp16, kind="ExternalInput")
    iotac = nc.dram_tensor("iotac", [P, 1], fp16, kind="ExternalInput")
    w3 = nc.dram_tensor("w3", [641, 256], fp16, kind="ExternalInput")
    w2 = nc.dram_tensor("w2", [129, 256], fp16, kind="ExternalInput")
    gbe3 = nc.dram_tensor("gbe3", [4, 256], f32, kind="ExternalInput")
    gbe2 = nc.dram_tensor("gbe2", [4, 128], f32, kind="ExternalInput")
    gidx_i = nc.dram_tensor("gidx_i", [Q, P, T_CAP // 16], i16, kind="ExternalInput")
    gidx_j = nc.dram_tensor("gidx_j", [Q, P, T_CAP // 16], i16, kind="ExternalInput")
    gidx_k = nc.dram_tensor("gidx_k", [Q, P, T_CAP // 16], i16, kind="ExternalInput")
    gkj32 = nc.dram_tensor("gkj32", [Q, P, T_CH], i32, kind="ExternalInput")
    jiv = nc.dram_tensor("jiv", [Q, T_CH, P], fp16, kind="ExternalInput")
    cidx_i = nc.dram_tensor("cidx_i", [Q, P, T_CAP // 16], i16, kind="ExternalInput")
    cidx_j = nc.dram_tensor("cidx_j", [Q, P, T_CAP // 16], i16, kind="ExternalInput")
    out = nc.dram_tensor("out", [Q * T_CAP, DE], fp16, kind="ExternalOutput")
    c3s = [[nc.dram_tensor(f"c3s_{q}_{b}", [B_T, DE], fp16)
            for b in range(NBLK)] for q in range(Q)]

    with tile.TileContext(nc) as tc:
        with tc.tile_pool(name="const", bufs=1) as cp:
            ident = cp.tile([P, P], fp16)
            make_identity(nc, ident[:])
            ones1 = cp.tile([1, P], fp16)
            nc.vector.memset(ones1[:], 1.0)
            epst = cp.tile([P, 1], f32)
            nc.vector.memset(epst[:], EPS)
            w3t = []
            for kc in range(5):
                wt = cp.tile([P, 256], fp16, name=f"w3_{kc}")
                nc.sync.dma_start(out=wt[:], in_=w3[kc * P:(kc + 1) * P, :])
                w3t.append(wt)
            w3b = cp.tile([1, 256], fp16, name="w3b")
            nc.sync.dma_start(out=w3b[:], in_=w3[640:641, :])
            w2t = cp.tile([P, 256], fp16, name="w2t")
            nc.sync.dma_start(out=w2t[:], in_=w2[0:128, :])
            w2b = cp.tile([1, 256], fp16, name="w2b")
            nc.sync.dma_start(out=w2b[:], in_=w2[128:129, :])
            if not fast:
                g3 = cp.tile([P, 256], f32)
                be3 = cp.tile([P, 256], f32)
                g2 = cp.tile([P, 256], f32)
                be2 = cp.tile([P, 256], f32)
                g32 = cp.tile([P, 128], f32)
                be32 = cp.tile([P, 128], f32)
                g22 = cp.tile([P, 128], f32)
                be22 = cp.tile([P, 128], f32)
                for tl, src in ((g2, gbe3[0]), (be2, gbe3[1]),
                                (g3, gbe3[2]), (be3, gbe3[3]),
                                (g22, gbe2[0]), (be22, gbe2[1]),
                                (g32, gbe2[2]), (be32, gbe2[3])):
                    nc.gpsimd.dma_start(out=tl[:], in_=bass.AP(
                        tensor=src.tensor, offset=src.offset,
                        ap=[[0, P], src.ap[0]]))
            iotaf_t = cp.tile([P, P], fp16)
            nc.sync.dma_start(out=iotaf_t[:], in_=iotaf[:, :])
            iotac_t = cp.tile([P, 1], fp16)
            nc.sync.dma_start(out=iotac_t[:], in_=iotac[:, :])

            # ---- interleaved phases ----
            with tc.tile_pool(name="abig", bufs=2) as abig, \
                 tc.tile_pool(name="aidx", bufs=2) as aidx, \
                 tc.tile_pool(name="asm", bufs=3) as asm, \
                 tc.tile_pool(name="aps", bufs=2, space="PSUM") as aps:
                bbig, bidx, bsm, bps = abig, aidx, asm, aps

                def emit_A_block(q, b):
                    if True:
                        cc = B_T // 16
                        xts = []
                        for nm, src in (("i", gidx_i), ("j", gidx_j),
                                        ("k", gidx_k)):
                            it = aidx.tile([P, cc], i16, tag=f"ix{nm}")
                            nc.sync.dma_start(
                                out=it[:], in_=src[q, :, b * cc:(b + 1) * cc])
                            xt = abig.tile([P, 1, B_T], fp16, tag=f"xT{nm}")
                            nc.gpsimd.dma_gather(
                                out_ap=xt[:], in_ap=node_h[:, :], idxs_ap=it[:],
                                num_idxs=B_T, num_idxs_reg=B_T,
                                elem_size=DN, transpose=True,
                                single_packet=False)
                            xts.append(xt)
                        kjt = aidx.tile([P, B_CH], i32, tag="kj32")
                        nc.sync.dma_start(
                            out=kjt[:],
                            in_=gkj32[q, :, b * B_CH:(b + 1) * B_CH])
                        ekr = abig.tile([P, B_CH, P], fp16, tag="ekr")
                        for c in range(B_CH):
                            nc.gpsimd.indirect_dma_start(
                                out=ekr[:, c, :], out_offset=None,
                                in_=edge_h[:, :],
                                in_offset=bass.IndirectOffsetOnAxis(
                                    ap=kjt[:, c:c + 1], axis=0))
                        ept = abig.tile([P, B_CH, P], fp16, tag="ept")
                        nc.sync.dma_start(
                            out=ept[:],
                            in_=eprimeT[q, :, b * B_T:(b + 1) * B_T]
                            .rearrange("f (n p) -> f n p", p=P))
                        jit = aidx.tile([P, B_CH], fp16, tag="jit")
                        nc.sync.dma_start(
                            out=jit[:], in_=jiv[q, b * B_CH:(b + 1) * B_CH]
                            .rearrange("c p -> p c"))
                        jsrc = jiv[q, b * B_CH:(b + 1) * B_CH]
                        jwt = abig.tile([P, B_CH, P], fp16, tag="jwt")
                        nc.gpsimd.dma_start(out=jwt[:], in_=bass.AP(
                            tensor=jsrc.tensor, offset=jsrc.offset,
                            ap=[[0, P], [1, B_T]]))
                        zs = abig.tile([P, B_CH, 256], fp16, tag="zs")
                        mvg = abig.tile([P, 2 * B_CH], f32, tag="mvg")
                        msum = abig.tile([P, B_CH, P], fp16, tag="msum")
                        for c in range(B_CH):
                            cs = slice(c * P, (c + 1) * P)
                            tps = aps.tile([P, P], fp16, tag="tps")
                            nc.tensor.transpose(tps[:], ekr[:, c, :], ident[:])
                            ekT = asm.tile([P, P], fp16, tag="ekT")
                            nc.vector.tensor_copy(ekT[:], tps[:])
                            yw = aps.tile([P, 256], f32, tag="yw")
                            nc.tensor.matmul(yw[:], lhsT=ept[:, c, :],
                                             rhs=w3t[3][:],
                                             start=True, stop=True)
                            yws = asm.tile([P, 256], fp16, tag="yws")
                            nc.scalar.copy(yws[:], yw[:])
                            selT = asm.tile([P, P], fp16, tag="selT")
                            nc.vector.tensor_tensor(
                                out=selT[:], in0=iotac_t[:].to_broadcast([P, P]),
                                in1=jwt[:, c, :], op=OP.is_equal)
                            z = aps.tile([P, 256], f32, tag="z")
                            for si in range(3):
                                nc.tensor.matmul(z[:], lhsT=xts[si][:, 0, cs],
                                                 rhs=w3t[si][:],
                                                 start=(si == 0), stop=False)
                            nc.tensor.matmul(z[:], lhsT=ekT[:], rhs=w3t[4][:],
                                             start=False, stop=False)
                            nc.tensor.matmul(z[:], lhsT=selT[:], rhs=yws[:],
                                             start=False, stop=fast)
                            if not fast:
                                nc.tensor.matmul(z[:], lhsT=ones1[:],
                                                 rhs=w3b[:],
                                                 start=False, stop=True)
                            stats = asm.tile([P, 6], f32, tag="stats")
                            nc.vector.bn_stats(stats[:], z[:])
                            nc.vector.bn_aggr(mvg[:, 2 * c:2 * c + 2], stats[:])
                            nc.scalar.copy(zs[:, c, :], z[:])
                        sd = asm.tile([P, B_CH], f32, tag="sd")
                        nc.scalar.activation(sd[:], mvg[:, 1::2], AF.Sqrt,
                                             bias=epst[:], scale=1.0)
                        rs = asm.tile([P, B_CH], f32, tag="rs")
                        nc.vector.reciprocal(rs[:], sd[:])
                        nm_t = asm.tile([P, B_CH], f32, tag="nm")
                        nc.vector.scalar_tensor_tensor(
                            out=nm_t[:], in0=mvg[:, 0::2], scalar=-1.0,
                            in1=rs[:], op0=OP.mult, op1=OP.mult)
                        for c in range(B_CH):
                            sg = asm.tile([P, P], f32, tag="sg")
                            th = asm.tile([P, P], f32, tag="th")
                            if fast:
                                nc.scalar.activation(
                                    sg[:], zs[:, c, 0:128], AF.Sigmoid,
                                    bias=nm_t[:, c:c + 1],
                                    scale=rs[:, c:c + 1])
                                nc.scalar.activation(
                                    th[:], zs[:, c, 128:256], AF.Tanh,
                                    bias=nm_t[:, c:c + 1],
                                    scale=rs[:, c:c + 1])
                            else:
                                nrm = asm.tile([P, 256], f32, tag="nrm")
                                nc.vector.tensor_scalar(
                                    out=nrm[:], in0=zs[:, c, :],
                                    scalar1=mvg[:, 2 * c:2 * c + 1],
                                    scalar2=rs[:, c:c + 1],
                                    op0=OP.subtract, op1=OP.mult)
                                nc.vector.tensor_mul(nrm[:], nrm[:], g3[:])
                                nc.vector.tensor_add(nrm[:], nrm[:], be3[:])
                                nc.scalar.activation(sg[:], nrm[:, 0:128],
                                                     AF.Sigmoid)
                                nc.scalar.activation(th[:], nrm[:, 128:256],
                                                     AF.Tanh)
                            msg = asm.tile([P, P], fp16, tag="msg")
                            nc.vector.tensor_mul(msg[:], sg[:], th[:])
                            jbc = jit[:, c:c + 1].to_broadcast([P, P])
                            pmap = asm.tile([P, P], fp16, tag="pmap")
                            nc.vector.tensor_tensor(
                                out=pmap[:], in0=jbc, in1=iotaf_t[:],
                                op=OP.is_equal)
                            mmt = aps.tile([P, 128], f32, tag="mm")
                            mm = mmt[:, :]
                            nc.tensor.matmul(mm, lhsT=pmap[:],
                                             rhs=msg[:], start=True, stop=True)
                            nc.scalar.copy(msum[:, c, :], mm)
                        nc.sync.dma_start(
                            out=c3s[q][b][:, :].rearrange(
                                "(n p) f -> p n f", p=P), in_=msum[:])

                def emit_B_block(q, c0, nch):
                    if True:
                        ne = nch * P
                        e0 = c0 * P
                        cc = ne // 16
                        msgt = bbig.tile([P, nch, P], fp16, tag="msgt")
                        nc.sync.dma_start(
                            out=msgt[:],
                            in_=c3s[q][c0 // B_CH][:, :]
                            .rearrange("(n p) f -> p n f", p=P))
                        nT = []
                        for nm, src in (("i", cidx_i), ("j", cidx_j)):
                            it = bidx.tile([P, cc], i16, tag=f"bix{nm}")
                            nc.sync.dma_start(
                                out=it[:], in_=src[q, :, c0 * 8:c0 * 8 + cc])
                            xt = bbig.tile([P, 1, ne], fp16, tag=f"bnT{nm}")
                            nc.gpsimd.dma_gather(
                                out_ap=xt[:], in_ap=node_h[:, :], idxs_ap=it[:],
                                num_idxs=ne, num_idxs_reg=ne,
                                elem_size=DN, transpose=True,
                                single_packet=False)
                            nT.append(xt)
                        edt = bbig.tile([P, nch, P], fp16, tag="edt")
                        nc.sync.dma_start(
                            out=edt[:],
                            in_=eprime[q, e0:e0 + ne, :]
                            .rearrange("(n p) f -> p n f", p=P))
                        z2s = bbig.tile([P, nch, 256], fp16, tag="z2s")
                        mv2g = bbig.tile([P, 2 * nch], f32, tag="mv2g")
                        for c in range(nch):
                            cs = slice(c * P, (c + 1) * P)
                            prodt = bsm.tile([P, P], fp16, tag="prodt")
                            nc.vector.tensor_mul(prodt[:], nT[0][:, 0, cs],
                                                 nT[1][:, 0, cs])
                            z2 = bps.tile([P, 256], f32, tag="z")
                            nc.tensor.matmul(z2[:], lhsT=prodt[:], rhs=w2t[:],
                                             start=True, stop=fast)
                            if not fast:
                                nc.tensor.matmul(z2[:], lhsT=ones1[:],
                                                 rhs=w2b[:],
                                                 start=False, stop=True)
                            st2 = bsm.tile([P, 6], f32, tag="st2")
                            nc.vector.bn_stats(st2[:], z2[:])
                            nc.vector.bn_aggr(mv2g[:, 2 * c:2 * c + 2], st2[:])
                            nc.scalar.copy(z2s[:, c, :], z2[:])
                        sd2 = bsm.tile([P, nch], f32, tag="sd2")
                        nc.scalar.activation(sd2[:], mv2g[:, 1::2], AF.Sqrt,
                                             bias=epst[:], scale=1.0)
                        rs2 = bsm.tile([P, nch], f32, tag="rs2")
                        nc.vector.reciprocal(rs2[:], sd2[:])
                        nm2 = bsm.tile([P, nch], f32, tag="nm2")
                        nc.vector.scalar_tensor_tensor(
                            out=nm2[:], in0=mv2g[:, 0::2], scalar=-1.0,
                            in1=rs2[:], op0=OP.mult, op1=OP.mult)
                        c2ps = bbig.tile([P, nch, P], fp16, tag="c2ps")
                        mvcg = bbig.tile([P, 2 * nch], f32, tag="mvcg")
                        mvmg = bbig.tile([P, 2 * nch], f32, tag="mvmg")
                        for c in range(nch):
                            sg2 = bsm.tile([P, P], f32, tag="sg2")
                            th2 = bsm.tile([P, P], f32, tag="th2")
                            if fast:
                                nc.scalar.activation(
                                    sg2[:], z2s[:, c, 0:128], AF.Sigmoid,
                                    bias=nm2[:, c:c + 1],
                                    scale=rs2[:, c:c + 1])
                                nc.scalar.activation(
                                    th2[:], z2s[:, c, 128:256], AF.Tanh,
                                    bias=nm2[:, c:c + 1],
                                    scale=rs2[:, c:c + 1])
                            else:
                                nrm2 = bsm.tile([P, 256], f32, tag="nrm2")
                                nc.vector.tensor_scalar(
                                    out=nrm2[:], in0=z2s[:, c, :],
                                    scalar1=mv2g[:, 2 * c:2 * c + 1],
                                    scalar2=rs2[:, c:c + 1],
                                    op0=OP.subtract, op1=OP.mult)
                                nc.vector.tensor_mul(nrm2[:], nrm2[:], g2[:])
                                nc.vector.tensor_add(nrm2[:], nrm2[:], be2[:])
                                nc.scalar.activation(sg2[:], nrm2[:, 0:128],
                                                     AF.Sigmoid)
                                nc.scalar.activation(th2[:], nrm2[:, 128:256],
                                                     AF.Tanh)
                            nc.vector.tensor_mul(c2ps[:, c, :], sg2[:], th2[:])
                            stc = bsm.tile([P, 6], f32, tag="stc")
                            nc.vector.bn_stats(stc[:], c2ps[:, c, :])
                            nc.vector.bn_aggr(mvcg[:, 2 * c:2 * c + 2], stc[:])
                            stm = bsm.tile([P, 6], f32, tag="stm")
                            nc.vector.bn_stats(stm[:], msgt[:, c, :])
                            nc.vector.bn_aggr(mvmg[:, 2 * c:2 * c + 2], stm[:])
                        sdc = bsm.tile([P, 2 * nch], f32, tag="sdc")
                        nc.scalar.activation(sdc[:, :nch], mvcg[:, 1::2],
                                             AF.Sqrt, bias=epst[:], scale=1.0)
                        nc.scalar.activation(sdc[:, nch:], mvmg[:, 1::2],
                                             AF.Sqrt, bias=epst[:], scale=1.0)
                        rsc = bsm.tile([P, 2 * nch], f32, tag="rsc")
                        nc.vector.reciprocal(rsc[:], sdc[:])
                        outt = bbig.tile([P, nch, P], fp16, tag="outt")
                        for c in range(nch):
                            c2e = bsm.tile([P, P], f32, tag="c2e")
                            nc.vector.tensor_scalar(
                                out=c2e[:], in0=c2ps[:, c, :],
                                scalar1=mvcg[:, 2 * c:2 * c + 1],
                                scalar2=rsc[:, c:c + 1],
                                op0=OP.subtract, op1=OP.mult)
                            if not fast:
                                nc.vector.tensor_mul(c2e[:], c2e[:], g22[:])
                                nc.vector.tensor_add(c2e[:], c2e[:], be22[:])
                            c3e = bsm.tile([P, P], f32, tag="c3e")
                            nc.vector.tensor_scalar(
                                out=c3e[:], in0=msgt[:, c, :],
                                scalar1=mvmg[:, 2 * c:2 * c + 1],
                                scalar2=rsc[:, nch + c:nch + c + 1],
                                op0=OP.subtract, op1=OP.mult)
                            if not fast:
                                nc.vector.tensor_mul(c3e[:], c3e[:], g32[:])
                                nc.vector.tensor_add(c3e[:], c3e[:], be32[:])
                            acc = bsm.tile([P, P], f32, tag="acc")
                            nc.vector.tensor_add(acc[:], c2e[:], c3e[:])
                            nc.vector.tensor_add(acc[:], acc[:], edt[:, c, :])
                            nc.scalar.activation(outt[:, c, :], acc[:], AF.Tanh)
                        nc.sync.dma_start(
                            out=out[q * T_CAP + e0:q * T_CAP + e0 + ne, :]
                            .rearrange("(n p) f -> p n f", p=P), in_=outt[:])

                B_OFFS = [(k * B_CH, B_CH) for k in range(NBLK)]
                for q in range(Q):
                    for b in range(NBLK):
                        emit_A_block(q, b)
                        if b >= 1:
                            c0b, nchb = B_OFFS[b - 1]
                            emit_B_block(q, c0b, nchb)
                    c0b, nchb = B_OFFS[NBLK - 1]
                    emit_B_block(q, c0b, nchb)
    nc.finalize()
    return nc


def kernel(**inputs):
    from concourse.bass_utils import run_bass_kernel_spmd

    i = np.asarray(inputs["i"]).astype(np.int64)
    j = np.asarray(inputs["j"]).astype(np.int64)
    idx_i = np.asarray(inputs["index_i"]).astype(np.int64)
    idx_j = np.asarray(inputs["index_j"]).astype(np.int64)
    idx_k = np.asarray(inputs["index_k"]).astype(np.int64)
    ji = np.asarray(inputs["index_ji"]).astype(np.int64)
    kj = np.asarray(inputs["index_kj"]).astype(np.int64)
    node = np.asarray(inputs["node_embedding"], np.float32)
    edge = np.asarray(inputs["edge_embedding"], np.float32)

    node_h = node.astype(np.float16)
    edge_h = edge.astype(np.float16)
    w3f = np.vstack([np.asarray(inputs["w_c3"], np.float32),
                     np.asarray(inputs["b_c3"], np.float32)[None]])
    w2f = np.vstack([np.asarray(inputs["w_c2"], np.float32),
                     np.asarray(inputs["b_c2"], np.float32)[None]])
    w3h = w3f.astype(np.float16)
    w2h = w2f.astype(np.float16)
    gbe3 = np.stack([np.asarray(inputs["g_bn_c2"], np.float32),
                     np.asarray(inputs["be_bn_c2"], np.float32),
                     np.asarray(inputs["g_bn_c3"], np.float32),
                     np.asarray(inputs["be_bn_c3"], np.float32)])
    gbe2 = np.stack([np.asarray(inputs["g_bn_c2_2"], np.float32),
                     np.asarray(inputs["be_bn_c2_2"], np.float32),
                     np.asarray(inputs["g_bn_c3_2"], np.float32),
                     np.asarray(inputs["be_bn_c3_2"], np.float32)])
    fast = (np.all(gbe3[0] == 1) and np.all(gbe3[2] == 1)
            and np.all(gbe2[0] == 1) and np.all(gbe2[2] == 1)
            and np.all(gbe3[1] == 0) and np.all(gbe3[3] == 0)
            and np.all(gbe2[1] == 0) and np.all(gbe2[3] == 0)
            and np.all(w3f[640] == 0) and np.all(w2f[128] == 0))

    order = np.argsort(ji, kind="stable")
    ji_sorted = ji[order]

    iotaf = np.tile(np.arange(P, dtype=np.float16)[None, :], (P, 1))
    iotac = np.arange(P, dtype=np.float16)[:, None]
    in_maps = []
    emaps = []
    for m in range(N_CORES):
        d, emap = _prep_core(m, i, j, idx_i, idx_j, idx_k, ji, kj,
                             order, ji_sorted)
        emaps.append(emap)
        epr = np.zeros((Q, T_CAP, DE), np.float16)
        for q in range(Q):
            base = m * E_SH + E_OFF[q]
            val = emap[q] >= 0
            epr[q][val] = edge_h[base + emap[q][val]]
        eprT = np.ascontiguousarray(epr.transpose(0, 2, 1))
        d.update(node_h=node_h, edge_h=edge_h, eprime=epr, eprimeT=eprT,
                 iotaf=iotaf, iotac=iotac,
                 w3=w3h, w2=w2h, gbe3=gbe3, gbe2=gbe2)
        in_maps.append(d)

    key = ("k3", fast)
    if key not in _CACHE:
        _CACHE[key] = _build_kernel(fast)
    nc = _CACHE[key]

    import os
    trace = bool(os.environ.get("KERNEL_TRACE"))
    res = run_bass_kernel_spmd(nc, in_maps, core_ids=list(range(N_CORES)),
                               trace=trace)
    global LAST_RESULT
    LAST_RESULT = res

    full = np.zeros((N_EDGES, DE), np.float32)
    for m in range(N_CORES):
        o = np.asarray(res.results[m]["out"], np.float32)
        emap = emaps[m]
        for q in range(Q):
            base = m * E_SH + E_OFF[q]
            val = emap[q] >= 0
            full[base + emap[q][val]] = o[q * T_CAP:(q + 1) * T_CAP][val]
    return full
